# revision 12
# baseline (speedup 1.0000x reference)
"""Trainium2 Bass kernel for nn_Attention_40407052320883 (sparse GQA attention).

Sharding: B(2) x KV(4) = 8 independent attention problems, one per NeuronCore.
Each core computes, for its (batch b, kv-group g):
  - qT/kT/vT projections (weights stationary, x^T moving, fp16 matmuls)
  - RMSNorm via ones-matmul partition reduction + fused Rsqrt; RoPE via
    swap-matrix matmul; all elementwise work in fp16 (DVE 2x mode), PSUM
    evacuations on the Pool engine
  - transposed-S attention: S^T = K Q^T, softcap -> exp (bias -4 keeps the
    unnormalized weights inside fp16 range; no max pass needed since tanh
    bounds logits to +-50), sliding-window blocks only, multiplicative edge
    masks; denominator via DVE fp16 accumulation of the exp blocks plus one
    ones-matmul per (j,h); PV with v stationary produces ctx^T directly;
    normalization on Pool
  - out-projection in transposed space: out^T = Wo_r^T ctx^T, fp16 result
Host: transposes x, slices weights, builds RoPE tables (with q/k norm scales
folded in), sums the 4 per-kv partial out^T per batch and transposes back.
"""

import numpy as np

B, S, E = 2, 2048, 2048
H, KV, D = 16, 4, 128
G = H // KV
WIN = 1024
CAP = 50.0
EPS = 1e-6
THETA = 10000.0
SCALE = D ** -0.5

N_CORES = 8
EC = E // 128          # 16 e-chunks
ST = S // 128          # 16 s-tiles
NQ = S // 512          # 4 s-quarters
EXP_BIAS = -4.0        # exp(cap*tanh - 4): keeps p in fp16 range both ways

# sliding-window block geometry: for q-chunk j (512 wide) and k-block m (128
# wide), d0 = 4j - m.  full blocks: 1<=d0<=4.  partial causal: -3<=d0<=0.
# partial window: 5<=d0<=8.  column ranges (within the 512-wide q chunk) that
# can contain nonzero weights, widened so narrow pairs share a range:
_D0_RANGE = {
    -3: (256, 512), -2: (256, 512), -1: (128, 512), 0: (0, 512),
    5: (0, 512), 6: (0, 384), 7: (0, 256), 8: (0, 256),
}
_D0_MASK_IDX = {-3: 0, -2: 1, -1: 2, 0: 3, 5: 4, 6: 5, 7: 6, 8: 7}


def _build_module(nrep=1):
    import contextlib
    import concourse.bacc as bacc
    import concourse.tile as tile
    import concourse.mybir as mybir

    f32 = mybir.dt.float32
    f16 = mybir.dt.float16
    MUL = mybir.AluOpType.mult
    ADD = mybir.AluOpType.add
    Act = mybir.ActivationFunctionType

    nc = bacc.Bacc(
        "TRN2", target_bir_lowering=False, debug=False, enable_asserts=False,
        num_devices=N_CORES,
    )

    xT = nc.dram_tensor("xT", [EC, 128, S], f16, kind="ExternalInput").ap()
    wqkv = nc.dram_tensor("wqkv", [EC, 128, 768], f16, kind="ExternalInput").ap()
    wo = nc.dram_tensor("wo", [G, 128, E], f16, kind="ExternalInput").ap()
    ctq = nc.dram_tensor("ctq", [128, S], f16, kind="ExternalInput").ap()
    stq = nc.dram_tensor("stq", [128, S], f16, kind="ExternalInput").ap()
    ctk = nc.dram_tensor("ctk", [128, S], f16, kind="ExternalInput").ap()
    stk = nc.dram_tensor("stk", [128, S], f16, kind="ExternalInput").ap()
    masks = nc.dram_tensor("masks", [8, 128, 512], f16, kind="ExternalInput").ap()
    ones = nc.dram_tensor("ones", [128, 1], f16, kind="ExternalInput").ap()
    ident = nc.dram_tensor("ident", [128, 128], f16, kind="ExternalInput").ap()
    swap = nc.dram_tensor("swap", [128, 128], f16, kind="ExternalInput").ap()
    outT = nc.dram_tensor("outT", [E, S], f16, kind="ExternalOutput").ap()

    c1 = float(SCALE / CAP)

    with tile.TileContext(nc) as tc:
      with nc.allow_low_precision(
              reason="fp16 softmax-denominator accumulation: <=12 adds of "
                     "positive values, ~0.2% worst-case on den"):
       with (tc.For_i(0, nrep, 1) if nrep > 1 else contextlib.nullcontext()):
        with (
            tc.tile_pool(name="consts", bufs=1) as consts,
            tc.tile_pool(name="mask", bufs=1) as m_pool,
            tc.tile_pool(name="qkv", bufs=1) as qkv_pool,
        ):
            mask_sb = m_pool.tile([128, 8, 512], f16, tag="masks")
            ones_sb = consts.tile([128, 1], f16, tag="ones")
            eps_sb = consts.tile([1, 1], f32, tag="eps")
            nc.gpsimd.memset(eps_sb[:, :], float(EPS))
            ebias_sb = consts.tile([128, 1], f32, tag="ebias")
            nc.gpsimd.memset(ebias_sb[:, :], float(EXP_BIAS))
            ident_sb = consts.tile([128, 128], f16, tag="ident")
            swap_sb = consts.tile([128, 128], f16, tag="swap")
            nc.sync.dma_start(ones_sb[:, :], ones[:, :])
            nc.sync.dma_start(ident_sb[:, :], ident[:, :])
            nc.sync.dma_start(swap_sb[:, :], swap[:, :])

            qT_sb = qkv_pool.tile([128, G, S], f16, tag="qT")
            kT_sb = qkv_pool.tile([128, S], f16, tag="kT")
            v_sb = qkv_pool.tile([128, ST, 128], f16, tag="v")

            # ---------------- phase 1: projections + rmsnorm + rope ---------
            with (
                tc.tile_pool(name="wq", bufs=1) as w_pool,
                tc.tile_pool(name="xq", bufs=3) as x_pool,
                tc.tile_pool(name="tab", bufs=1) as tab_pool,
                tc.tile_pool(name="p1t", bufs=2) as t_pool,
                tc.tile_pool(name="p1v", bufs=1) as vt_pool,
                tc.tile_pool(name="p1ps", bufs=3, space="PSUM") as ps1,
                tc.tile_pool(name="p1ps2", bufs=1, space="PSUM") as ps1b,
                tc.tile_pool(name="p1ps3", bufs=2, space="PSUM") as ps1c,
            ):
                wq_sb = w_pool.tile([128, EC, 768], f16, tag="wqkv")

                for qt in range(NQ):
                    sl = slice(qt * 512, (qt + 1) * 512)
                    xq = []
                    for ec in range(EC):
                        # interleave weight + activation loads so the first
                        # accumulation chain can start as soon as chunk 0 lands
                        if qt == 0:
                            nc.sync.dma_start(
                                wq_sb[:, ec, 0:256], wqkv[ec, :, 0:256])
                            nc.sync.dma_start(
                                wq_sb[:, ec, 256:768], wqkv[ec, :, 256:768])
                        t = x_pool.tile([128, 512], f16, tag=f"xq{ec}")
                        nc.sync.dma_start(t[:, :], xT[ec, :, sl])
                        xq.append(t)
                    if qt == 1:
                        # masks are not needed until attention starts; load
                        # them after the startup-critical projection inputs
                        for i in range(8):
                            nc.sync.dma_start(
                                mask_sb[:, i, :], masks[i, :, :])
                    ctq_t = tab_pool.tile([128, 512], f16, tag="ctq")
                    stq_t = tab_pool.tile([128, 512], f16, tag="stq")
                    ctk_t = tab_pool.tile([128, 512], f16, tag="ctk")
                    stk_t = tab_pool.tile([128, 512], f16, tag="stk")
                    nc.sync.dma_start(ctq_t[:, :], ctq[:, sl])
                    nc.sync.dma_start(stq_t[:, :], stq[:, sl])
                    nc.sync.dma_start(ctk_t[:, :], ctk[:, sl])
                    nc.sync.dma_start(stk_t[:, :], stk[:, sl])

                    for ch in range(6):
                        ps = ps1.tile([128, 512], f32, tag="pqkv")
                        for ec in range(EC):
                            nc.tensor.matmul(
                                ps[:, :],
                                wq_sb[:, ec, ch * 128:(ch + 1) * 128],
                                xq[ec][:, :],
                                start=(ec == 0), stop=(ec == EC - 1),
                            )
                        if ch == 5:
                            # v: evacuate + transpose back to [s, d]
                            vt = vt_pool.tile([128, 512], f16, tag="vT")
                            nc.scalar.copy(vt[:, :], ps[:, :])
                            for t4 in range(4):
                                tr = ps1c.tile([128, 128], f16, tag="tr")
                                nc.tensor.transpose(
                                    tr[:, :], vt[:, t4 * 128:(t4 + 1) * 128],
                                    ident_sb[:, :])
                                nc.vector.tensor_copy(
                                    v_sb[:, qt * 4 + t4, :], tr[:, :])
                        else:
                            # rmsnorm: sum of squares over d via ones-matmul,
                            # 1/rms via fused Rsqrt, scale folded into q
                            # before rope (per-column scale commutes w/ rope)
                            sq = t_pool.tile([128, 512], f16, tag="sq")
                            nc.scalar.activation(sq[:, :], ps[:, :], Act.Square)
                            qc = t_pool.tile([128, 512], f16, tag="qc")
                            nc.vector.tensor_copy(qc[:, :], ps[:, :])
                            var = ps1b.tile([1, 512], f32, tag="var")
                            nc.tensor.matmul(
                                var[:, :], ones_sb[:, :], sq[:, :],
                                start=True, stop=True)
                            sd = t_pool.tile([1, 512], f32, tag="sd")
                            nc.scalar.activation(
                                sd[:, :], var[:, :], Act.Sqrt,
                                bias=eps_sb[:, :], scale=float(1.0 / D))
                            rr = t_pool.tile([1, 512], f16, tag="rr")
                            nc.vector.reciprocal(rr[:, :], sd[:, :])
                            rnb = t_pool.tile([128, 512], f16, tag="rnb")
                            nc.gpsimd.partition_broadcast(rnb[:, :], rr[:, :])
                            qn = t_pool.tile([128, 512], f16, tag="qn")
                            nc.vector.tensor_tensor(
                                qn[:, :], qc[:, :], rnb[:, :], op=MUL)
                            qsw_ps = ps1c.tile([128, 512], f32, tag="qsw")
                            nc.tensor.matmul(
                                qsw_ps[:, :], swap_sb[:, :], qn[:, :],
                                start=True, stop=True)
                            qsw = t_pool.tile([128, 512], f16, tag="qsws")
                            nc.vector.tensor_copy(qsw[:, :], qsw_ps[:, :])
                            ct_t, st_t = (ctq_t, stq_t) if ch < 4 else (ctk_t, stk_t)
                            t1 = t_pool.tile([128, 512], f16, tag="t1")
                            t2 = t_pool.tile([128, 512], f16, tag="t2")
                            nc.vector.tensor_tensor(
                                t1[:, :], qn[:, :], ct_t[:, :], op=MUL)
                            nc.vector.tensor_tensor(
                                t2[:, :], qsw[:, :], st_t[:, :], op=MUL)
                            dst = qT_sb[:, ch, sl] if ch < 4 else kT_sb[:, sl]
                            nc.vector.tensor_tensor(
                                dst, t1[:, :], t2[:, :], op=ADD)

            # ---------------- phase 2: attention ----------------------------
            with (
                tc.tile_pool(name="wo", bufs=1) as wo_pool,
                tc.tile_pool(name="ctx", bufs=1) as ctx_pool,
            ):
                ctx_sb = ctx_pool.tile([128, G, S], f16, tag="ctx")
                wo_sb = wo_pool.tile([128, G, E], f16, tag="wo")
                for g in range(G):
                    nc.sync.dma_start(wo_sb[:, g, :], wo[g, :, :])

                with (
                    tc.tile_pool(name="p2t", bufs=2) as a_pool,
                    tc.tile_pool(name="p2o", bufs=3) as ob_pool,
                ):
                  with (
                    tc.tile_pool(name="p2ps", bufs=2, space="PSUM") as st_pool,
                    tc.tile_pool(name="p2ctx", bufs=2, space="PSUM") as ps_ctx,
                    tc.tile_pool(name="p2den", bufs=1, space="PSUM") as ps_den,
                    tc.tile_pool(name="p3ps", bufs=1, space="PSUM") as ps3,
                  ):
                      # out-projection units (ec, jj) interleaved into the
                      # attention loop to fill PE slack while ACT is saturated
                      pending = []

                      def emit_oproj(pool=None):
                          ec, jj = pending.pop(0)
                          esl = slice(ec * 128, (ec + 1) * 128)
                          po = (pool or ps3).tile([128, 512], f32, tag="po")
                          for hh in range(G):
                              nc.tensor.matmul(
                                  po[:, :], wo_sb[:, hh, esl],
                                  ctx_sb[:, hh, jj * 512:(jj + 1) * 512],
                                  start=(hh == 0), stop=(hh == G - 1))
                          ob = ob_pool.tile([128, 512], f16, tag="ob")
                          nc.vector.tensor_copy(ob[:, :], po[:, :])
                          nc.sync.dma_start(
                              outT[esl, jj * 512:(jj + 1) * 512], ob[:, :])

                      for j in range(NQ):
                          jsl = slice(j * 512, (j + 1) * 512)
                          m_lo, m_hi = max(0, 4 * j - 8), min(ST - 1, 4 * j + 3)
                          ms = list(range(m_lo, m_hi + 1))
                          # full-width blocks first so the start=True matmul
                          # covers the whole psum bank (uniform pending-zero)
                          ms = sorted(
                              ms, key=lambda m: _D0_RANGE.get(4 * j - m, (0, 512))
                              != (0, 512))
                          groups = [ms[i:i + 2] for i in range(0, len(ms), 2)]
                          for h in range(G):
                              ctx_ps = ps_ctx.tile([128, 512], f32, tag="ctx")
                              acc = a_pool.tile([128, 512], f16, tag="acc")
                              first = True
                              n_m = len(ms)
                              done = 0
                              for grp in groups:
                                  gw = len(grp) * 512
                                  st_ps = st_pool.tile([128, 1024], f32, tag="st")
                                  p_sb = a_pool.tile([128, 1024], f16, tag="p")
                                  t_sb = a_pool.tile([128, 1024], f32, tag="t")
                                  rngs = [(m, 4 * j - m) +
                                          _D0_RANGE.get(4 * j - m, (0, 512))
                                          for m in grp]
                                  # when both blocks of a pair share the same
                                  # narrowed range, write only that range and
                                  # run tanh/exp on a [128, 2, w] strided view
                                  rset = {(w0, w1) for (_, _, w0, w1) in rngs}
                                  narrow = (len(grp) == 2 and len(rset) == 1
                                            and rset != {(0, 512)})
                                  for k, (m, d0, w0, w1) in enumerate(rngs):
                                      s0, s1 = (w0, w1) if narrow else (0, 512)
                                      nc.tensor.matmul(
                                          st_ps[:, k * 512 + s0:k * 512 + s1],
                                          kT_sb[:, m * 128:(m + 1) * 128],
                                          qT_sb[:, h,
                                                j * 512 + s0:j * 512 + s1],
                                          start=True, stop=True)
                                  if pending:
                                      emit_oproj()
                                  if narrow:
                                      w0, w1 = rngs[0][2], rngs[0][3]
                                      st_v = st_ps[:, :].rearrange(
                                          "p (k f) -> p k f", k=2)[:, :, w0:w1]
                                      t_v = t_sb[:, :].rearrange(
                                          "p (k f) -> p k f", k=2)[:, :, w0:w1]
                                      p_v = p_sb[:, :].rearrange(
                                          "p (k f) -> p k f", k=2)[:, :, w0:w1]
                                      nc.scalar.activation(
                                          t_v, st_v, Act.Tanh, scale=c1)
                                      nc.scalar.activation(
                                          p_v, t_v, Act.Exp, scale=float(CAP),
                                          bias=ebias_sb[:, :])
                                  else:
                                      nc.scalar.activation(
                                          t_sb[:, :gw], st_ps[:, :gw],
                                          Act.Tanh, scale=c1)
                                      nc.scalar.activation(
                                          p_sb[:, :gw], t_sb[:, :gw],
                                          Act.Exp, scale=float(CAP),
                                          bias=ebias_sb[:, :])
                                  for k, (m, d0, w0, w1) in enumerate(rngs):
                                      psl = slice(k * 512 + w0, k * 512 + w1)
                                      if d0 in _D0_MASK_IDX:
                                          mi = _D0_MASK_IDX[d0]
                                          nc.vector.tensor_tensor(
                                              p_sb[:, psl], p_sb[:, psl],
                                              mask_sb[:, mi, w0:w1], op=MUL)
                                      done += 1
                                      last = done == n_m
                                      nc.tensor.matmul(
                                          ctx_ps[:, w0:w1],
                                          v_sb[:, m, :], p_sb[:, psl],
                                          start=first, stop=last)
                                      if first:
                                          nc.gpsimd.tensor_copy(
                                              acc[:, :], p_sb[:, 0:512])
                                      else:
                                          nc.gpsimd.tensor_tensor(
                                              acc[:, w0:w1], acc[:, w0:w1],
                                              p_sb[:, psl], op=ADD)
                                      first = False
                              den_ps = ps_den.tile([1, 512], f32, tag="den")
                              nc.tensor.matmul(
                                  den_ps[:, :], ones_sb[:, :], acc[:, :],
                                  start=True, stop=True)
                              rec_sb = a_pool.tile([1, 512], f16, tag="rec")
                              nc.vector.reciprocal(rec_sb[:, :], den_ps[:, :])
                              rb2 = a_pool.tile([128, 512], f16, tag="rb2")
                              nc.gpsimd.partition_broadcast(rb2[:, :], rec_sb[:, :])
                              nc.vector.tensor_tensor(
                                  ctx_sb[:, h, jsl], ctx_ps[:, :], rb2[:, :],
                                  op=MUL)
                          # enqueue this j-chunk's out-projection units
                          pending.extend((ec, j) for ec in range(EC))
                  # attention pools released: drain the tail with deeper
                  # psum buffering
                  with tc.tile_pool(name="p3ps2", bufs=3, space="PSUM") as ps3b:
                      while pending:
                          emit_oproj(ps3b)

    nc.compile()
    return nc


def _host_tables(positions_b, scale_vec):
    """cos/sin tables in [d, s] layout with norm-scale folded in, signed sin."""
    half = D // 2
    inv_freq = (1.0 / (THETA ** (np.arange(half, dtype=np.float32) / half))
                ).astype(np.float32)
    ang = positions_b.astype(np.float32)[:, None] * inv_freq[None, :]  # [S,64]
    cos = np.cos(ang).astype(np.float32)  # [S, 64]
    sin = np.sin(ang).astype(np.float32)
    sc = scale_vec.astype(np.float32)
    ct = np.empty((128, S), np.float32)
    st = np.empty((128, S), np.float32)
    ct[:half] = (cos * sc[None, :half]).T
    ct[half:] = (cos * sc[None, half:]).T
    st[:half] = (-sin * sc[None, half:]).T
    st[half:] = (sin * sc[None, :half]).T
    return ct.astype(np.float16), st.astype(np.float16)


def _host_masks():
    m = np.zeros((8, 128, 512), np.float32)
    ki = np.arange(128)[:, None]
    qf = np.arange(512)[None, :]
    for d0, idx in _D0_MASK_IDX.items():
        dist = 128 * d0 + qf - ki
        m[idx] = ((dist >= 0) & (dist < WIN)).astype(np.float32)
    return m.astype(np.float16)


_NC_CACHE = {}


def _get_module(nrep=1):
    key = f"nc{nrep}"
    if key not in _NC_CACHE:
        _NC_CACHE[key] = _build_module(nrep)
    return _NC_CACHE[key]


def _core_inputs(x, positions, Wq, Wk, Wv, Wo, q_norm_scale, k_norm_scale):
    masks_np = _host_masks()
    ones_np = np.ones((128, 1), np.float16)
    ident_np = np.eye(128, dtype=np.float16)
    swap_np = np.roll(np.eye(128, dtype=np.float16), 64, axis=0)

    per_b = {}
    for b in range(B):
        xT_np = np.ascontiguousarray(x[b].T).reshape(EC, 128, S).astype(np.float16)
        ctq_np, stq_np = _host_tables(positions[b], q_norm_scale)
        ctk_np, stk_np = _host_tables(positions[b], k_norm_scale)
        per_b[b] = (xT_np, ctq_np, stq_np, ctk_np, stk_np)

    in_maps = []
    for c in range(N_CORES):
        b, kv = c // KV, c % KV
        xT_np, ctq_np, stq_np, ctk_np, stk_np = per_b[b]
        wq_slice = Wq[:, kv * G:(kv + 1) * G, :].reshape(E, G * D)
        wk_slice = Wk[:, kv, :]
        wv_slice = Wv[:, kv, :]
        wqkv_np = (np.concatenate([wq_slice, wk_slice, wv_slice], axis=1)
                   .reshape(EC, 128, 768).astype(np.float16))
        wo_np = np.ascontiguousarray(Wo[kv * G:(kv + 1) * G]).astype(np.float16)
        in_maps.append({
            "xT": xT_np, "wqkv": wqkv_np, "wo": wo_np,
            "ctq": ctq_np, "stq": stq_np, "ctk": ctk_np, "stk": stk_np,
            "masks": masks_np, "ones": ones_np, "ident": ident_np,
            "swap": swap_np,
        })
    return in_maps


def kernel(x, positions, mask, Wq, Wk, Wv, Wo, q_norm_scale, k_norm_scale,
           **_unused):
    from concourse import bass_utils

    x = np.asarray(x, np.float32)
    positions = np.asarray(positions)
    Wq = np.asarray(Wq, np.float32)
    Wk = np.asarray(Wk, np.float32)
    Wv = np.asarray(Wv, np.float32)
    Wo = np.asarray(Wo, np.float32)
    q_norm_scale = np.asarray(q_norm_scale, np.float32)
    k_norm_scale = np.asarray(k_norm_scale, np.float32)

    nc = _get_module()
    in_maps = _core_inputs(x, positions, Wq, Wk, Wv, Wo,
                           q_norm_scale, k_norm_scale)
    res = bass_utils.run_bass_kernel_spmd(
        nc, in_maps, core_ids=list(range(N_CORES)))
    out = np.zeros((B, S, E), np.float32)
    for c in range(N_CORES):
        b = c // KV
        out[b] += res.results[c]["outT"].astype(np.float32).T
    return out
